# revision 10
# baseline (speedup 1.0000x reference)
"""Distributed Conjugate Gradient solver on 8 Trainium2 NeuronCores.

Problem: X = CG_solve(M, RHS); M is [8192, 8192] SPD fp32; 20 iterations
with the reference's early-stop freeze semantics (state freezes once the
carried rTr <= 1e-10), emulated with a multiplicative gate.

Sharding: column-shard of M (hint's row-shard + local matvec + AllGather,
expressed via M^T so DMA tiles are natural row-major). Core i holds
MsT_i = M[i*S:(i+1)*S, :].T (shape [n, S], S = n/8). Each iteration:

  y_i = MsT_i.T @ p    (contracts over the FULL p -> [S] slice of M @ p,
                        since M is symmetric; fixed AP offsets -> SPMD-safe)
  AllGather(y_i) -> Ap on every core (the only collective per iteration)
  dot products / axpy updates done redundantly per core (vectors are tiny).

Vector layout on-chip: "row-chunk" [64, 128] tiles (partition c holds
elements c*128..(c+1)*128). The matvec needs p column-major [128, 64]
(k-tile g = column g), produced each iteration with one PE transpose.

Matmul precision modes (PE fp32 streams at 1/4 rate, so full fp32 is
PE-bound; DMA of the 32MB/core M shard is the target bottleneck):
  "f32"  - exact fp32 matmuls (4 cyc/row)
  "b3"   - M split hi/lo into bf16 on the HOST (same total DMA bytes);
           Ap ~= Mhi@p_hi + Mhi@p_lo + Mlo@p_hi (3 bf16 matmuls, 1 cyc/row,
           matvec rel err ~2^-18)
  "f32r" - TF32-like reduced precision (1 cyc/row, rel err ~1.4e-4)
"""

import sys
import numpy as np

if "/opt/trn_rl_repo" not in sys.path:
    sys.path.insert(0, "/opt/trn_rl_repo")

N = 8192
NCORES = 8
NITER = 20
TOL = 1e-10

MM_MODE = "b3"   # "f32" | "b3" | "f32r"
KPACK = 4        # k-tiles of 128 packed per DMA
MBUFS = 8        # M-tile pool depth (per precision stream)

_cache = {}


def build(n=N, ncores=NCORES, niter=NITER, mm_mode=MM_MODE, kpack=KPACK,
          mbufs=MBUFS):
    import concourse.bacc as bacc
    import concourse.mybir as mybir
    from concourse import tile

    f32 = mybir.dt.float32
    bf16 = mybir.dt.bfloat16
    f32r = mybir.dt.float32r
    shard = n // ncores
    VP = n // 128                   # vector-tile partitions
    assert VP <= 128 and n % 128 == 0
    KT = n // 128                   # k-tiles (contraction tiles / p_cm cols)
    MM_N = 512                      # moving free dim (PSUM bank = 512 fp32)
    NBW = min(shard, 1024)          # n-block width (bounds live PSUM banks)
    NB = shard // NBW               # n-blocks (1 at the real 8-core geometry)
    NS = NBW // MM_N                # matmuls per k-tile per stream per block
    KB = KT // kpack                # DMA blocks per n-block
    assert KT % kpack == 0 and shard % MM_N == 0 and shard % NBW == 0

    nc = bacc.Bacc(num_devices=ncores)

    if mm_mode == "b3":
        Mhi = nc.dram_tensor("Mhi", [n, shard], bf16, kind="ExternalInput")
        Mlo = nc.dram_tensor("Mlo", [n, shard], bf16, kind="ExternalInput")
        m_streams = [Mhi, Mlo]
    else:
        mdt = f32r if mm_mode == "f32r" else f32
        Ms = nc.dram_tensor("MsT", [n, shard], mdt, kind="ExternalInput")
        m_streams = [Ms]
    RHS = nc.dram_tensor("RHS", [n], f32, kind="ExternalInput")
    X = nc.dram_tensor("X", [n], f32, kind="ExternalOutput")

    # NB: 1-D DRAM tensors / degenerate 1-D APs on the y-DMA made NEFF
    # loading fail on this runtime; keep these 2-D.
    y_dram = nc.dram_tensor("y_loc", [1, shard], f32)
    ap_dram = nc.dram_tensor("ap_full", [ncores, shard], f32, addr_space="Shared")

    m_views = [t[:, :].rearrange("(t p) j -> t p j", p=128) for t in m_streams]
    RHS_rc = RHS[:].rearrange("(c r) -> c r", r=128)
    X_rc = X[:].rearrange("(c r) -> c r", r=128)
    ap_rc_v = ap_dram[:, :].rearrange("a (c r) -> (a c) r", r=128)

    add, mult = mybir.AluOpType.add, mybir.AluOpType.mult
    is_gt, is_eq = mybir.AluOpType.is_gt, mybir.AluOpType.is_equal

    with tile.TileContext(nc) as tc:
        with (
            tc.tile_pool(name="const", bufs=1) as cpool,
            tc.tile_pool(name="vec", bufs=1) as vpool,
            tc.tile_pool(name="mtiles", bufs=mbufs) as mpool,
            tc.tile_pool(name="ps_acc", bufs=2, space="PSUM") as ps_acc,
            tc.tile_pool(name="ps_misc", bufs=1, space="PSUM") as ps_misc,
        ):
            # ---- constants ----
            ones_t = cpool.tile([VP, 128], f32, tag="ones")
            nc.vector.memset(ones_t[:], 1.0)

            # ---- persistent state ----
            x_rc = vpool.tile([VP, 128], f32, tag="x")
            r_rc = vpool.tile([VP, 128], f32, tag="r")
            p_rc = vpool.tile([VP, 128], f32, tag="p")
            ap_rc = vpool.tile([VP, 128], f32, tag="ap")
            scr_rc = vpool.tile([VP, 128], f32, tag="scr")
            y_sb = vpool.tile([1, shard], f32, tag="ysb")

            p_cm = vpool.tile([128, KT], f32, tag="pcm")
            if mm_mode == "b3":
                p_hi = vpool.tile([128, KT], bf16, tag="phi")
                p_lo = vpool.tile([128, KT], bf16, tag="plo")
                p_err = vpool.tile([128, KT], f32, tag="perr")
            elif mm_mode == "f32r":
                p_r = vpool.tile([128, KT], f32r, tag="pr")

            scr2_rc = vpool.tile([VP, 128], f32, tag="scr2")
            rtr_t = vpool.tile([128, 1], f32, tag="rtr")
            g_t = vpool.tile([128, 1], f32, tag="g")
            omg_t = vpool.tile([128, 1], f32, tag="omg")      # 1 - gate
            rog_t = vpool.tile([128, 1], f32, tag="rog")      # gate / rtr_old
            alpha_t = vpool.tile([128, 1], f32, tag="alpha")
            alphan_t = vpool.tile([128, 1], f32, tag="alphan")
            beta_t = vpool.tile([128, 1], f32, tag="beta")
            recip_t = vpool.tile([128, 1], f32, tag="recip")
            part_t = vpool.tile([VP, 1], f32, tag="part")
            part2_t = vpool.tile([VP, 1], f32, tag="part2")

            def dot_to(ps_col, a, b, part):
                """ps_col[128,1] = sum(a*b), broadcast to all 128 partitions."""
                nc.vector.scalar_tensor_tensor(
                    scr2_rc[:], a[:], 1.0, b[:], op0=mult, op1=mult,
                    accum_out=part[:])
                nc.tensor.matmul(ps_col, ones_t[:], part[:], start=True, stop=True)

            def gate_precompute():
                """Next iteration's gate terms; runs off the critical path
                (overlaps the next matvec). rtr_t must hold the carried rTr."""
                nc.vector.tensor_single_scalar(g_t[:], rtr_t[:], TOL, op=is_gt)
                nc.vector.tensor_scalar(
                    omg_t[:], g_t[:], -1.0, 1.0, op0=mult, op1=add)
                nc.vector.reciprocal(recip_t[:], rtr_t[:])
                nc.vector.tensor_mul(rog_t[:], recip_t[:], g_t[:])

            def make_p_views():
                """Transpose p -> column-major and derive precision views."""
                for bi in range(VP // 32):
                    for bj in range(4):
                        nc.vector.transpose(
                            p_cm[bj * 32:(bj + 1) * 32, bi * 32:(bi + 1) * 32],
                            p_rc[bi * 32:(bi + 1) * 32, bj * 32:(bj + 1) * 32])
                if mm_mode == "b3":
                    nc.vector.tensor_copy(p_hi[:], p_cm[:])
                    nc.vector.tensor_sub(p_err[:], p_cm[:], p_hi[:])
                    nc.vector.tensor_copy(p_lo[:], p_err[:])
                elif mm_mode == "f32r":
                    nc.vector.tensor_copy(p_r[:], p_cm[:])

            # ---- init: r = RHS; p = r; x = 0; rtr = r.r; p views ----
            nc.sync.dma_start(r_rc[:], RHS_rc[:])
            nc.vector.tensor_copy(p_rc[:], r_rc[:])
            nc.vector.memset(x_rc[:], 0.0)

            dots_ps = ps_misc.tile([128, 2], f32, tag="dots")
            dot_to(dots_ps[:, 1:2], r_rc, r_rc, part_t)
            nc.vector.tensor_copy(rtr_t[:], dots_ps[:, 1:2])
            gate_precompute()
            make_p_views()

            mdt_tile = {"b3": bf16, "f32r": f32r, "f32": f32}[mm_mode]

            for it in range(niter):
                # ---- matvec: y_local = sum_g p[g-tile] . M[g-tile, :] ----
                for nb in range(NB):
                    y_ps = [ps_acc.tile([1, MM_N], f32,
                                        name=f"yps{it}_{nb}_{s}", tag=f"yps{s}")
                            for s in range(NS)]
                    for kb in range(KB):
                        mts = []
                        for si, mv in enumerate(m_views):
                            mt = mpool.tile([128, kpack, NBW], mdt_tile, tag=f"mt{si}")
                            nc.sync.dma_start(
                                mt[:],
                                mv[kb * kpack:(kb + 1) * kpack,
                                   :, nb * NBW:(nb + 1) * NBW]
                                .rearrange("t p j -> p t j"))
                            mts.append(mt)
                        for t in range(kpack):
                            g = kb * kpack + t
                            first, last = (g == 0), (g == KT - 1)
                            for s in range(NS):
                                sl = slice(s * MM_N, (s + 1) * MM_N)
                                if mm_mode == "b3":
                                    nc.tensor.matmul(
                                        y_ps[s][:], p_hi[:, g:g + 1], mts[0][:, t, sl],
                                        start=first, stop=False)
                                    nc.tensor.matmul(
                                        y_ps[s][:], p_lo[:, g:g + 1], mts[0][:, t, sl],
                                        start=False, stop=False)
                                    nc.tensor.matmul(
                                        y_ps[s][:], p_hi[:, g:g + 1], mts[1][:, t, sl],
                                        start=False, stop=last)
                                elif mm_mode == "f32r":
                                    nc.tensor.matmul(
                                        y_ps[s][:], p_r[:, g:g + 1], mts[0][:, t, sl],
                                        start=first, stop=last)
                                else:
                                    nc.tensor.matmul(
                                        y_ps[s][:], p_cm[:, g:g + 1], mts[0][:, t, sl],
                                        start=first, stop=last)
                    for s in range(NS):
                        nc.scalar.copy(
                            y_sb[:, nb * NBW + s * MM_N:nb * NBW + (s + 1) * MM_N],
                            y_ps[s][:])

                # ---- AllGather y -> Ap (SWDGE lanes: isolated from the
                # M-prefetch DMAHW sems, else the trigger waits ~11us) ----
                nc.gpsimd.dma_start(y_dram[:, :], y_sb[:, :])
                nc.gpsimd.collective_compute(
                    "AllGather", mybir.AluOpType.bypass,
                    replica_groups=[list(range(ncores))],
                    ins=[y_dram[:]], outs=[ap_dram[:]])
                nc.sync.dma_start(ap_rc[:], ap_rc_v[:])

                # ---- critical scalar chain (g/omg/rog precomputed) ----
                dots_ps = ps_misc.tile([128, 2], f32, tag="dots")
                dot_to(dots_ps[:, 0:1], p_rc, ap_rc, part_t)    # pTAp
                nc.vector.reciprocal(alphan_t[:], dots_ps[:, 0:1])
                nc.vector.tensor_scalar(                        # alpha = g*rtr/pAp
                    alpha_t[:], alphan_t[:], rtr_t[:], g_t[:], op0=mult, op1=mult)
                nc.vector.tensor_scalar_mul(alphan_t[:], alpha_t[:], -1.0)

                nc.vector.scalar_tensor_tensor(                 # r -= alpha Ap
                    r_rc[:], ap_rc[:], alphan_t[:VP, :], r_rc[:], op0=mult, op1=add)
                dot_to(dots_ps[:, 1:2], r_rc, r_rc, part2_t)    # rnTrn
                nc.vector.tensor_scalar(                        # beta_g
                    beta_t[:], dots_ps[:, 1:2], rog_t[:], omg_t[:],
                    op0=mult, op1=add)
                # p = beta_g * p + gate * r   (x update is off-path: uses
                # the pre-update p, so emit it before p is overwritten but
                # after the critical r/rn chain)
                nc.vector.tensor_single_scalar(scr_rc[:], r_rc[:], g_t[:VP, :], op=mult)
                nc.vector.scalar_tensor_tensor(                 # x += alpha p
                    x_rc[:], p_rc[:], alpha_t[:VP, :], x_rc[:], op0=mult, op1=add)
                nc.vector.scalar_tensor_tensor(
                    p_rc[:], p_rc[:], beta_t[:VP, :], scr_rc[:], op0=mult, op1=add)

                if it < niter - 1:
                    make_p_views()

                # ---- off-critical-path updates (overlap next matvec) ----
                nc.vector.tensor_copy(rtr_t[:], dots_ps[:, 1:2])
                gate_precompute()

            nc.sync.dma_start(X_rc[:], x_rc[:])

    nc.compile()
    return nc


def get_nc(**kw):
    key = tuple(sorted(kw.items()))
    if key not in _cache:
        _cache[key] = build(**kw)
    return _cache[key]


def shard_inputs(M, RHS, n=N, ncores=NCORES, mm_mode=MM_MODE):
    """Host-side sharding. Core i gets M[i*S:(i+1)*S, :].T contiguous
    (for "b3", split into bf16 hi + lo)."""
    import ml_dtypes
    shard = n // ncores
    rhs = np.ascontiguousarray(RHS, dtype=np.float32)
    in_maps = []
    for i in range(ncores):
        slab = np.ascontiguousarray(M[i * shard:(i + 1) * shard, :].T)
        if mm_mode == "b3":
            hi = slab.astype(ml_dtypes.bfloat16)
            lo = (slab - hi.astype(np.float32)).astype(ml_dtypes.bfloat16)
            in_maps.append({"Mhi": hi, "Mlo": lo, "RHS": rhs})
        else:
            in_maps.append({"MsT": slab, "RHS": rhs})
    return in_maps


def kernel(X, M, RHS):
    from concourse.bass_utils import run_bass_kernel_spmd

    nc = get_nc()
    in_maps = shard_inputs(np.asarray(M, dtype=np.float32),
                           np.asarray(RHS, dtype=np.float32))
    res = run_bass_kernel_spmd(nc, in_maps, core_ids=list(range(NCORES)))
    return res.results[0]["X"].astype(np.float32)
